# revision 1
# baseline (speedup 1.0000x reference)
# Trainium2 Bass kernel for nn_Adapter_Router_plus (moe_routing).
#
# Reference computation (per batch sample b):
#   w   = softmax((x[0] @ We.T + be) / T)                      # [E]
#   y_e = silu(x @ Wa[e].T + ba[e])                            # [N, H]
#   z_e = grouped_conv1x1(y_e, Wb[e]) + bb[e]                  # [N, C]
#   out = sum_e w_e * z_e + x
#
# Sharding: pure data-parallel over B=8 across the 8 NeuronCores (one
# sample per core, no collectives).  Weights are replicated.
#
# Per-core dataflow (N=2048 tokens, C=4096), processed in token groups
# (slab = 128 tokens; schedule [2,4,4,4,2] slabs — small first/last groups
# shorten pipeline fill and the tail drain):
#   - x slabs [128, 4096] stream in via SWDGE cast-DMA (f32 HBM -> bf16 SBUF)
#   - each 128x128 block is transposed on the TensorEngine via a regular
#     bf16 matmul against an identity (lhsT = x-block, rhs = I, stays
#     HAM-warm unlike transpose-mode), ACT copy-casts PSUM -> bf16 SBUF
#   - A-proj: yT[(g,e,h'), n] accumulates over 32 c-chunks (E*H = 128 rows
#     exactly fills the partition dim)
#   - router: 32 tiny accumulating matmuls -> softmax on one partition ->
#     broadcast via DRAM bounce to a [128,1] per-row scale -> fold into Wb
#   - B-proj back to natural [n, c] layout with K=128 (zero-padded per
#     group); bb folded in via a rank-2 matmul only when bb != 0; each
#     group's B-proj is deferred and interleaved into the next group's
#     chunk loop so the PE never stalls on the DVE PSUM drain
#   - DVE adds the residual (mixed f32-PSUM + bf16-SBUF), output is
#     written bf16 (host upcast is exact) to halve the store traffic

import numpy as np
import ml_dtypes

B, N, C = 8, 2048, 4096
E, H, G = 2, 64, 2
SCALE, T = 1.0, 10.0
HG, CG = H // G, C // G   # 32, 2048
P = 128
CK = C // P               # 32 contraction chunks
GROUP_SLABS = 4           # slabs (of 128 tokens) per group -> 512-token groups

BF16 = ml_dtypes.bfloat16

_PROGRAM_CACHE = {}

# scheduling/buffering knobs (sweepable via analyze_sweep)
TUNE = {
    "xb_bufs": 12, "xt_bufs": 8, "yw_bufs": 2, "out_bufs": 4,
    "pt_bufs": 2, "py_bufs": 1, "pz_bufs": 3,
    # model-ablation flags (analysis only)
    "ablate_in": False, "ablate_out": False, "ablate_z": False, "ablate_ta": False,
    "swdge_queues": 1, "router_own_bank": False, "split_copies": False,
    "small_edges": True, "sched16": None, "in_split": 1, "z_pair": False,
    "pair_chunks": True, "pair_io": False, "pair_out": False, "late_wb": False,
}


def _pack_weights(Wa, ba, Wb, bb, We, be):
    """Host-side marshalling of the (tiny) weights into kernel layouts.

    Row layout on the chip: r = g*64 + e*32 + h'  with  h = g*32 + h'.
    """
    Wa = np.asarray(Wa, np.float32)
    ba = np.asarray(ba, np.float32)
    Wb = np.asarray(Wb, np.float32)
    bb = np.asarray(bb, np.float32)
    We = np.asarray(We, np.float32)
    be = np.asarray(be, np.float32)

    # Wa [E, H, C] -> Wa_pack [r, c] -> wa dram tile [p, k*P + m] = Wa_pack[m, k*P+p]
    Wa_pack = Wa.reshape(E, G, HG, C).transpose(1, 0, 2, 3).reshape(P, C)
    wa_host = np.ascontiguousarray(
        Wa_pack.T.reshape(CK, P, P).transpose(1, 0, 2).reshape(P, CK * P)
    ).astype(BF16)

    ba_host = np.ascontiguousarray(
        ba.reshape(E, G, HG).transpose(1, 0, 2).reshape(P, 1)
    ).astype(np.float32)

    # Wb [E, G, CG, HG] -> zero-padded [128, C]: rows of g-block are non-zero
    # only in that g's output columns, so the B matmul can contract K=128.
    wb_host = np.zeros((P, C), np.float32)
    for g in range(G):
        blk = Wb[:, g].transpose(0, 2, 1).reshape(E * HG, CG)  # [(e,h'), c']
        wb_host[g * 64:(g + 1) * 64, g * CG:(g + 1) * CG] = blk
    wb_host = wb_host.astype(BF16)

    # We [E, C] -> we dram tile [p, k*2 + e] = We[e, k*P + p]
    we_host = np.ascontiguousarray(
        We.T.reshape(CK, P, E).transpose(1, 0, 2).reshape(P, CK * E)
    ).astype(BF16)

    be_host = np.ascontiguousarray(be.reshape(1, E)).astype(np.float32)

    bb_host = np.ascontiguousarray(bb.reshape(E, C)).astype(BF16)
    with_bb = bool(np.any(bb != 0.0))

    ident_host = np.eye(P, dtype=np.float32).astype(BF16)

    return {
        "wa": wa_host, "ba": ba_host, "wb": wb_host, "we": we_host,
        "be": be_host, "bbp": bb_host, "ident": ident_host,
        "ones2": np.ones((E, P), np.float32).astype(BF16),
    }, with_bb, bool(np.any(ba != 0.0))


def _build_program(n_tokens, with_bb, with_ba):
    """Build (and compile) the per-core Bacc program."""
    import concourse.bass as bass
    import concourse.mybir as mybir
    import concourse.tile as tile
    from concourse import bacc

    f32 = mybir.dt.float32
    bf16 = mybir.dt.bfloat16

    slabs = n_tokens // P
    assert slabs % GROUP_SLABS == 0
    if TUNE["sched16"] and slabs == 16:
        sched = list(TUNE["sched16"])
        assert sum(sched) == slabs
    elif TUNE["small_edges"] and slabs >= 6 and (slabs - 4) % GROUP_SLABS == 0:
        # small first/last groups: faster pipeline fill, half-size tail drain
        sched = [2] + [GROUP_SLABS] * ((slabs - 4) // GROUP_SLABS) + [2]
    else:
        sched = [GROUP_SLABS] * (slabs // GROUP_SLABS)

    nc = bacc.Bacc("TRN2", target_bir_lowering=False, debug=False, num_devices=1,
                   enable_partition_id=False, num_swdge_queues=TUNE["swdge_queues"])

    x_d = nc.dram_tensor("x", [n_tokens, C], f32, kind="ExternalInput").ap()
    wa_d = nc.dram_tensor("wa", [P, CK * P], bf16, kind="ExternalInput").ap()
    ba_d = nc.dram_tensor("ba", [P, 1], f32, kind="ExternalInput").ap()
    wb_d = nc.dram_tensor("wb", [P, C], bf16, kind="ExternalInput").ap()
    we_d = nc.dram_tensor("we", [P, CK * E], bf16, kind="ExternalInput").ap()
    be_d = nc.dram_tensor("be", [1, E], f32, kind="ExternalInput").ap()
    id_d = nc.dram_tensor("ident", [P, P], bf16, kind="ExternalInput").ap()
    if with_bb:
        bb_d = nc.dram_tensor("bbp", [E, C], bf16, kind="ExternalInput").ap()
        ones_d = nc.dram_tensor("ones2", [E, P], bf16, kind="ExternalInput").ap()
    out_d = nc.dram_tensor("out", [n_tokens, C], bf16, kind="ExternalOutput").ap()

    # paired-IO tiles are twice the size; tag slots size to the max tile,
    # so halve the buf counts to keep the same SBUF footprint
    xb_bufs = max(2, TUNE["xb_bufs"] // 2) if TUNE["pair_io"] else TUNE["xb_bufs"]
    out_bufs = max(2, TUNE["out_bufs"] // 2) if TUNE["pair_out"] else TUNE["out_bufs"]

    with tile.TileContext(nc) as tc:
        with (
            tc.tile_pool(name="wpool", bufs=1) as wpool,
            tc.tile_pool(name="dscratch", bufs=1, space="DRAM") as dram_pool,
            tc.tile_pool(name="xb", bufs=xb_bufs) as xb_pool,
            tc.tile_pool(name="xt", bufs=TUNE["xt_bufs"]) as xt_pool,
            tc.tile_pool(name="yw", bufs=TUNE["yw_bufs"]) as yw_pool,
            tc.tile_pool(name="outp", bufs=out_bufs) as out_pool,
            tc.tile_pool(name="pt", bufs=TUNE["pt_bufs"], space="PSUM") as psum_t,
            tc.tile_pool(name="py", bufs=TUNE["py_bufs"], space="PSUM") as psum_y,
            tc.tile_pool(name="pz", bufs=TUNE["pz_bufs"], space="PSUM") as psum_z,
        ):
            # ---- weights into SBUF ----
            wa_sb = wpool.tile([P, CK * P], bf16)
            nc.sync.dma_start(wa_sb[:], wa_d)
            wb_sb = wpool.tile([P, C], bf16)
            if not TUNE["late_wb"]:
                nc.sync.dma_start(wb_sb[:], wb_d)
            we_sb = wpool.tile([P, CK * E], bf16)
            nc.sync.dma_start(we_sb[:], we_d)
            ident = wpool.tile([P, P], bf16)
            nc.sync.dma_start(ident[:], id_d)
            ba_sb = wpool.tile([P, 1], f32)
            nc.sync.dma_start(ba_sb[:], ba_d)
            be_sb = wpool.tile([1, E], f32)
            nc.sync.dma_start(be_sb[:], be_d)
            if with_bb:
                bb_sb = wpool.tile([E, C], bf16)
                if not TUNE["late_wb"]:
                    nc.sync.dma_start(bb_sb[:], bb_d)

            # router results / scaled weights
            logits = wpool.tile([1, E], f32)
            rmax = wpool.tile([1, 1], f32)
            shifted = wpool.tile([1, E], f32)
            e_sb = wpool.tile([1, E], f32)
            rsum = wpool.tile([1, 1], f32)
            rinv = wpool.tile([1, 1], f32)
            w_sb = wpool.tile([1, E], f32)
            wvec = wpool.tile([P, 1], f32)
            wbs = wpool.tile([P, C], bf16)
            if with_bb:
                w2 = wpool.tile([E, 1], f32)
                bbw = wpool.tile([E, C], bf16)
                ones2 = wpool.tile([E, P], bf16)
                nc.sync.dma_start(ones2[:], ones_d)

            if TUNE["router_own_bank"]:
                pr_full = psum_t.tile([P, 512], f32, tag="pr", bufs=1)
            else:
                pr_full = psum_z.tile([P, 512], f32, tag="zt")
            pr_tile = pr_full[0:1, 0:E]


            # z-phase work of group g is deferred and interleaved into group
            # g+1's chunk loop so the PE never stalls on the DVE PSUM drain.
            pending = []

            def emit_pending(n=1):
                for _ in range(n):
                    if pending:
                        pending.pop(0)()

            base_slab = 0
            for g, gs in enumerate(sched):
                base = base_slab
                base_slab += gs
                # ---- load the group's slabs (cast f32 -> bf16 during DMA) ----
                xbs = []
                if TUNE["pair_io"] and gs % 2 == 0 and not TUNE["ablate_in"]:
                    # one 4 MiB cast-DMA per slab pair (better SDMA efficiency)
                    for j in range(gs // 2):
                        slab = base + 2 * j
                        xbp = xb_pool.tile([P, 2 * C], bf16, tag="xb",
                                           name=f"xbp_g{g}_{j}")
                        src_ap = x_d[slab * P:(slab + 2) * P, :].rearrange(
                            "(t p) c -> p t c", p=P)
                        nc.gpsimd.dma_start(xbp[:], src_ap)
                        xbs.append(xbp[:, 0:C])
                        xbs.append(xbp[:, C:2 * C])
                else:
                    for s in range(gs):
                        slab = base + s
                        xb = xb_pool.tile([P, C], bf16, tag="xb")
                        if TUNE["ablate_in"]:
                            nc.gpsimd.dma_start(xb[:, 0:P], x_d[slab * P:(slab + 1) * P, 0:P])
                        else:
                            nsp = TUNE["in_split"]
                            w = C // nsp
                            for q in range(nsp):
                                nc.gpsimd.dma_start(
                                    xb[:, q * w:(q + 1) * w],
                                    x_d[slab * P:(slab + 1) * P, q * w:(q + 1) * w])
                        xbs.append(xb)

                if g == 0 and TUNE["late_wb"]:
                    # wb/bb aren't needed until the first z-phase (a group
                    # later) — load them after the first slabs so they don't
                    # compete with pipeline fill
                    nc.sync.dma_start(wb_sb[:], wb_d)
                    if with_bb:
                        nc.sync.dma_start(bb_sb[:], bb_d)

                # ---- transpose chunks + A-proj accumulation ----
                py_tile = psum_y.tile([P, GROUP_SLABS * P], f32, name="py_t")[:, :gs * P]
                kstep = 2 if TUNE["pair_chunks"] else 1
                for k0 in range(0, CK if not TUNE["ablate_ta"] else 1, kstep):
                    # transpose kstep c-chunks into one PSUM tile, one ACT
                    # copy-cast for the pair, then one A-matmul per chunk
                    pt_full = psum_t.tile([P, kstep * GROUP_SLABS * P], f32, tag="pt", name="pt_t")
                    pt = pt_full[:, :kstep * gs * P]
                    for j in range(kstep):
                        for s in range(gs):
                            nc.tensor.matmul(
                                pt[:, (j * gs + s) * P:(j * gs + s + 1) * P],
                                lhsT=xbs[s][:, (k0 + j) * P:(k0 + j + 1) * P],
                                rhs=ident[:],
                                start=True, stop=True,
                            )
                    xt_full = xt_pool.tile([P, kstep * GROUP_SLABS * P], bf16, tag="xt", name="xt_t")
                    xt = xt_full[:, :kstep * gs * P]
                    nc.scalar.copy(xt[:], pt[:])
                    for j in range(kstep):
                        k = k0 + j
                        xtj = xt[:, j * gs * P:(j + 1) * gs * P]
                        nc.tensor.matmul(
                            py_tile[:],
                            lhsT=wa_sb[:, k * P:(k + 1) * P],
                            rhs=xtj[:],
                            start=(k == 0), stop=(k == CK - 1),
                        )
                        if g == 0:
                            # router logits: token 0 column of chunk k
                            nc.tensor.matmul(
                                pr_tile[:],
                                lhsT=xtj[:, 0:1],
                                rhs=we_sb[:, k * E:(k + 1) * E],
                                start=(k == 0), stop=(k == CK - 1),
                            )
                    emit_pending(kstep // 2 if kstep > 1 else (k0 % 2))

                if g == 0:
                    # ---- router softmax + weight folding ----
                    nc.vector.tensor_add(logits[:], pr_tile[:], be_sb[:])
                    nc.vector.reduce_max(rmax[:], logits[:], axis=mybir.AxisListType.X)
                    nc.vector.tensor_scalar_sub(shifted[:], logits[:], rmax[:])
                    nc.scalar.activation(
                        e_sb[:], shifted[:], mybir.ActivationFunctionType.Exp,
                        scale=1.0 / T,
                    )
                    nc.vector.reduce_sum(rsum[:], e_sb[:], axis=mybir.AxisListType.X)
                    nc.vector.reciprocal(rinv[:], rsum[:])
                    nc.vector.tensor_scalar_mul(w_sb[:], e_sb[:], rinv[:])
                    # broadcast w via a DRAM bounce -> per-row scale [(g,e,h'), 1]
                    wdram = dram_pool.tile([1, E], f32)
                    nc.sync.dma_start(wdram[:], w_sb[:])
                    for gg in range(G):
                        for e in range(E):
                            we_ap = wdram[0:1, e:e + 1]
                            src = bass.AP(
                                tensor=we_ap.tensor, offset=we_ap.offset,
                                ap=[[0, HG], [1, 1]],
                            )
                            r0 = gg * 64 + e * HG
                            nc.gpsimd.dma_start(wvec[r0:r0 + HG, :], src)
                    nc.vector.tensor_scalar_mul(wbs[:], wb_sb[:], wvec[:])
                    if with_bb:
                        # w on two partitions, then scale bb rows by it
                        nc.sync.dma_start(w2[:], wdram[:].rearrange("1 e -> e 1"))
                        nc.vector.tensor_scalar_mul(bbw[:], bb_sb[:], w2[:])

                # ---- silu (v * sigmoid(v)) + router-weighted B-proj ----
                yw = yw_pool.tile([P, GROUP_SLABS * P], bf16, tag="yw", name="yw_t")[:, :gs * P]
                sig = yw_pool.tile([P, GROUP_SLABS * P], f32, tag="sig", name="sig_t")[:, :gs * P]
                nc.scalar.activation(
                    sig[:], py_tile[:], mybir.ActivationFunctionType.Sigmoid,
                    bias=ba_sb[:], scale=1.0,
                )
                if with_ba:
                    v_sb = yw_pool.tile([P, GROUP_SLABS * P], f32, tag="vsb", name="vsb_t")[:, :gs * P]
                    nc.vector.tensor_scalar_add(v_sb[:], py_tile[:], ba_sb[:])
                    nc.vector.tensor_mul(yw[:], v_sb[:], sig[:])
                else:
                    nc.vector.tensor_mul(yw[:], py_tile[:], sig[:])

                out_tiles = {}

                pair_out = TUNE["pair_out"] and gs % 2 == 0
                out_pairs = {}

                def z_block(s, gg, yw=yw, xbs=xbs, g=g, out_tiles=out_tiles,
                            out_pairs=out_pairs, pair_out=pair_out):
                    if TUNE["ablate_z"]:
                        return
                    if gg == 0 and s not in out_tiles:
                        if pair_out:
                            j, t = s // 2, s % 2
                            if j not in out_pairs:
                                out_pairs[j] = out_pool.tile(
                                    [P, 2 * C], bf16, tag="out",
                                    name=f"outp_g{g}_{j}")
                            out_tiles[s] = out_pairs[j][:, t * C:(t + 1) * C]
                        else:
                            out_tiles[s] = out_pool.tile(
                                [P, C], bf16, tag="out", name=f"out_g{g}_s{s}")
                    out_sb = out_tiles[s]
                    if TUNE["z_pair"]:
                        for half in range(2):
                            zt = psum_z.tile([P, 1024], f32, tag="zt",
                                             name=f"zt_{half}")
                            for j in range(2):
                                col = gg * CG + (half * 2 + j) * 512
                                nc.tensor.matmul(
                                    zt[:, j * 512:(j + 1) * 512],
                                    lhsT=yw[:, s * P:(s + 1) * P],
                                    rhs=wbs[:, col:col + 512],
                                    start=True, stop=not with_bb,
                                )
                            if with_bb:
                                for j in range(2):
                                    col = gg * CG + (half * 2 + j) * 512
                                    nc.tensor.matmul(
                                        zt[:, j * 512:(j + 1) * 512], lhsT=ones2[:],
                                        rhs=bbw[:, col:col + 512],
                                        start=False, stop=True,
                                    )
                            col = gg * CG + half * 1024
                            nc.vector.tensor_add(
                                out_sb[:, col:col + 1024], zt[:],
                                xbs[s][:, col:col + 1024],
                            )
                    else:
                        zts = []
                        for cc in range(4):
                            col = gg * CG + cc * 512
                            zt = psum_z.tile([P, 512], f32, tag="zt")
                            zts.append(zt)
                            nc.tensor.matmul(
                                zt[:],
                                lhsT=yw[:, s * P:(s + 1) * P],
                                rhs=wbs[:, col:col + 512],
                                start=True, stop=not with_bb,
                            )
                        if with_bb:
                            for cc in range(4):
                                col = gg * CG + cc * 512
                                nc.tensor.matmul(
                                    zts[cc][:], lhsT=ones2[:],
                                    rhs=bbw[:, col:col + 512],
                                    start=False, stop=True,
                                )
                        for cc in range(4):
                            col = gg * CG + cc * 512
                            # residual add: out = z + x (mixed f32 psum + bf16 sbuf)
                            nc.vector.tensor_add(
                                out_sb[:, col:col + 512], zts[cc][:],
                                xbs[s][:, col:col + 512],
                            )

                def out_dma(s, base=base, out_tiles=out_tiles, xbs=xbs):
                    slab = base + s
                    out_sb = xbs[s] if TUNE["ablate_z"] else out_tiles[s]
                    if TUNE["ablate_out"]:
                        nc.sync.dma_start(out_d[slab * P:(slab + 1) * P, 0:P], out_sb[:, 0:P])
                    else:
                        nc.sync.dma_start(out_d[slab * P:(slab + 1) * P, :], out_sb[:])

                def out_dma_pair(j, base=base, out_pairs=out_pairs):
                    slab = base + 2 * j
                    dst = out_d[slab * P:(slab + 2) * P, :].rearrange(
                        "(t p) c -> p t c", p=P)
                    nc.sync.dma_start(dst, out_pairs[j][:])

                if pair_out and not TUNE["ablate_z"] and not TUNE["ablate_out"]:
                    for j in range(gs // 2):
                        for t in range(2):
                            pending.append(lambda s=2 * j + t: z_block(s, 0))
                            pending.append(lambda s=2 * j + t: z_block(s, 1))
                        pending.append(lambda j=j: out_dma_pair(j))
                else:
                    for s in range(gs):
                        pending.append(lambda s=s: z_block(s, 0))
                        pending.append(lambda s=s: z_block(s, 1))
                        pending.append(lambda s=s: out_dma(s))

            # drain the last group's z work
            emit_pending(len(pending))

    nc.compile()
    return nc


def _get_program(n_tokens, with_bb, with_ba):
    key = (n_tokens, with_bb, with_ba)
    if key not in _PROGRAM_CACHE:
        _PROGRAM_CACHE[key] = _build_program(n_tokens, with_bb, with_ba)
    return _PROGRAM_CACHE[key]


def _run(inputs, trace=False):
    from concourse import bass_utils

    x = np.asarray(inputs["x"], np.float32)
    weights, with_bb, with_ba = _pack_weights(
        inputs["Wa"], inputs["ba"], inputs["Wb"], inputs["bb"],
        inputs["We"], inputs["be"],
    )
    if not with_bb:
        weights = {k: v for k, v in weights.items() if k not in ("bbp", "ones2")}

    nc = _get_program(x.shape[1], with_bb, with_ba)

    in_maps = []
    for b in range(B):
        m = {"x": np.ascontiguousarray(x[b])}
        m.update(weights)
        in_maps.append(m)

    res = bass_utils.run_bass_kernel_spmd(
        nc, in_maps, core_ids=list(range(B)), trace=trace,
    )
    out = np.stack([r["out"] for r in res.results], axis=0)
    return out.astype(np.float32), res


def kernel(**inputs) -> np.ndarray:
    out, _ = _run(inputs, trace=False)
    return out

